# revision 1
# baseline (speedup 1.0000x reference)
"""Multi-head causal self-attention with RoPE on 8 Trainium2 NeuronCores.

Reference computation (B=2, S=2048, D=2048, H=16, DH=128):
    xs = hidden_q / sqrt(D)
    q,k,v = xs @ {Wq,Wk,Wv}.T        (reshaped to [B,H,S,DH])
    q,k <- RoPE(q,k)
    scores = q @ k.T / sqrt(DH)  (causal masked)
    p = softmax(scores); attn = p @ v
    out = (attn / sqrt(H*DH)) @ Wo.T

Sharding: 8 cores = 2 (batch) x 4 (head-groups of 4 heads).  Each core
computes its head-group's projections, attention and a partial output
projection; the host sums the 4 partials per batch.

All matmuls run in float32r (TF32-like, full PE rate at N=512).
Layouts on device (per core):
    xT   [D, S]    feature-major activations (host pre-transposed, pre-scaled)
    wqT  [D, 512]  per-group Wq slice, transposed
    scoresT [keys, queries] so softmax-denominators come from a ones-matmul
    attnT [dh, q] accumulated per head, normalized with broadcast reciprocal
    y    [S, D]    natural layout partial output (host sums over groups)
"""

import math
from contextlib import ExitStack

import numpy as np

import concourse.bass as bass
import concourse.mybir as mybir
import concourse.tile as tile
from concourse import bacc
from concourse.bass import ts
from concourse.bass_utils import run_bass_kernel_spmd
from concourse.masks import make_identity

B, S, D, H, DH = 2, 2048, 2048, 16, 128
BASE = 10000.0
G = 4              # head-groups (cores per batch)
HG = H // G        # heads per group = 4
F = HG * DH        # features per group = 512
NT = S // 128      # 16 token tiles
NQB = S // 512     # 4 query blocks
F32 = mybir.dt.float32
F32R = mybir.dt.float32r

_cache = {}


def _rope_tables():
    inv_freq = 1.0 / (BASE ** (np.arange(0, DH, 2, dtype=np.float64) / DH))
    t = np.arange(S, dtype=np.float64)
    freqs = np.outer(t, inv_freq)                       # [S, 64]
    return (np.cos(freqs).astype(np.float32), np.sin(freqs).astype(np.float32))


def _mask_tiles():
    # mask[o][j, q] = 1 if q >= j + 128*o else 0  (diagonal-band tiles)
    o = np.arange(4)[:, None, None]
    j = np.arange(128)[None, :, None]
    q = np.arange(512)[None, None, :]
    return (q >= j + 128 * o).astype(np.float32)        # [4, 128, 512]


def _build(reps=1):
    key = ("nc", reps)
    if key in _cache:
        return _cache[key]
    nc = bacc.Bacc("TRN2", target_bir_lowering=False, debug=False, num_devices=8)

    xT = nc.dram_tensor("xT", [D, S], F32R, kind="ExternalInput")
    wqT = nc.dram_tensor("wqT", [D, F], F32R, kind="ExternalInput")
    wkT = nc.dram_tensor("wkT", [D, F], F32R, kind="ExternalInput")
    wvT = nc.dram_tensor("wvT", [D, F], F32R, kind="ExternalInput")
    woT = nc.dram_tensor("woT", [F, D], F32R, kind="ExternalInput")
    cos_d = nc.dram_tensor("cos", [S, 64], F32R, kind="ExternalInput")
    sin_d = nc.dram_tensor("sin", [S, 64], F32R, kind="ExternalInput")
    msk_d = nc.dram_tensor("masks", [4, 128, 512], F32R, kind="ExternalInput")
    y = nc.dram_tensor("y", [S, D], F32, kind="ExternalOutput")

    # chunked spill tensors (one per 4-token-tile group) so phase-B reloads
    # depend only on their own chunk's spills, not the whole phase A
    q_spill = [nc.dram_tensor(f"q_spill{c}", [F, 512], F32R) for c in range(4)]
    k_spill = [nc.dram_tensor(f"k_spill{c}", [F, 512], F32R) for c in range(4)]
    q_spill_r = [t.ap().rearrange("(hb p) s -> p hb s", p=128) for t in q_spill]
    k_spill_r = [t.ap().rearrange("(hb p) s -> p hb s", p=128) for t in k_spill]

    xT_r = xT.ap().rearrange("(kt p) s -> p kt s", p=128)       # [128, 16, S]

    with tile.TileContext(nc) as tc, ExitStack() as ctx:
        const = ctx.enter_context(tc.tile_pool(name="const", bufs=1))
        vpool = ctx.enter_context(tc.tile_pool(name="vpool", bufs=1))
        ps512 = ctx.enter_context(tc.tile_pool(name="ps512", bufs=6, space="PSUM"))
        # transposes (phase A) and denominators (phase B) share slots
        ps_sm = ctx.enter_context(tc.tile_pool(name="ps_sm", bufs=2, space="PSUM"))

        ones_f = const.tile([128, 1], F32, tag="ones_f")
        nc.gpsimd.memset(ones_f[:], 1.0)
        ones = const.tile([128, 1], F32R, tag="ones")
        nc.vector.tensor_copy(ones[:], ones_f[:])
        ident_f = const.tile([128, 128], F32, tag="ident_f")
        make_identity(nc, ident_f[:])
        ident = const.tile([128, 128], F32R, tag="ident")
        nc.vector.tensor_copy(ident[:], ident_f[:])

        for _rep in range(reps):
            vh_cb = [vpool.tile([128, 4, F], F32R, tag=f"vh{c}", name=f"vh{c}") for c in range(4)]

            # ---------- Phase A: projections + RoPE + transpose + spill ----
            with ExitStack() as actx:
                wpool = actx.enter_context(tc.tile_pool(name="wpool", bufs=1))
                xpool = actx.enter_context(tc.tile_pool(name="xpool", bufs=3))
                rot_pool = actx.enter_context(tc.tile_pool(name="rot", bufs=2))
                tmp_pool = actx.enter_context(tc.tile_pool(name="tmp", bufs=4))
                stage = actx.enter_context(tc.tile_pool(name="stage", bufs=3))

                wq_sb = wpool.tile([128, NT, F], F32R, tag="wq")
                wk_sb = wpool.tile([128, NT, F], F32R, tag="wk")
                wv_sb = wpool.tile([128, NT, F], F32R, tag="wv")
                cos_sb = wpool.tile([128, NT, 64], F32R, tag="cos")
                sin_sb = wpool.tile([128, NT, 64], F32R, tag="sin")
                nc.sync.dma_start(cos_sb[:], cos_d.ap().rearrange("(t p) c -> p t c", p=128))
                nc.sync.dma_start(sin_sb[:], sin_d.ap().rearrange("(t p) c -> p t c", p=128))
                wqT_r = wqT.ap().rearrange("(kt p) f -> p kt f", p=128)
                wkT_r = wkT.ap().rearrange("(kt p) f -> p kt f", p=128)
                wvT_r = wvT.ap().rearrange("(kt p) f -> p kt f", p=128)
                # interleave x-tile prefetches into the weight stream so the
                # PE can chase the arriving weights through tb=0..2
                xq_tiles = {}
                for tb in range(3):
                    xq_tiles[tb] = xpool.tile([128, NT, 128], F32R, tag="xq", name=f"xq{tb}")
                nc.sync.dma_start(xq_tiles[0][:], xT_r[:, :, ts(0, 128)])
                for kt in range(NT):
                    nc.sync.dma_start(wq_sb[:, kt, :], wqT_r[:, kt, :])
                    nc.sync.dma_start(wk_sb[:, kt, :], wkT_r[:, kt, :])
                    nc.sync.dma_start(wv_sb[:, kt, :], wvT_r[:, kt, :])
                    if kt in (2, 5):
                        tb = 1 if kt == 2 else 2
                        nc.sync.dma_start(xq_tiles[tb][:], xT_r[:, :, ts(tb, 128)])

                for tb in range(NT):
                    if tb in xq_tiles:
                        xq = xq_tiles[tb]
                    else:
                        xq = xpool.tile([128, NT, 128], F32R, tag="xq")
                        nc.sync.dma_start(xq[:], xT_r[:, :, ts(tb, 128)])
                    pq = ps512.tile([128, 512], F32, tag="ps512")
                    pk = ps512.tile([128, 512], F32, tag="ps512")
                    pv = ps512.tile([128, 512], F32, tag="ps512")
                    for kt in range(NT):
                        f = dict(start=(kt == 0), stop=(kt == NT - 1))
                        nc.tensor.matmul(pq[:], xq[:, kt, :], wq_sb[:, kt, :], **f)
                        nc.tensor.matmul(pk[:], xq[:, kt, :], wk_sb[:, kt, :], **f)
                        nc.tensor.matmul(pv[:], xq[:, kt, :], wv_sb[:, kt, :], **f)
                    nc.vector.tensor_copy(vh_cb[tb // 4][:, tb % 4, :], pv[:])

                    # RoPE with broadcast APs: 4 wide DVE ops per tensor.
                    cos_b = cos_sb[:, tb, :].unsqueeze(1).unsqueeze(1) \
                        .broadcast_to((128, HG, 2, 64))
                    sin_b = sin_sb[:, tb, :].unsqueeze(1).broadcast_to((128, HG, 64))
                    for (ps, spill_r, rtag, stag) in (
                        (pq, q_spill_r, "qrot", "stq"),
                        (pk, k_spill_r, "krot", "stk"),
                    ):
                        ps_r = ps[:].rearrange("p (hb half j) -> p hb half j",
                                               hb=HG, half=2, j=64)
                        rot = rot_pool.tile([128, 512], F32R, tag=rtag)
                        rot_r = rot[:].rearrange("p (hb half j) -> p hb half j",
                                                 hb=HG, half=2, j=64)
                        tmp = tmp_pool.tile([128, HG, 2, 64], F32R, tag="tmp")
                        # tmp_lo = -q_hi * sin ; tmp_hi = +q_lo * sin
                        nc.vector.scalar_tensor_tensor(
                            tmp[:, :, 0, :], ps_r[:, :, 1, :], -1.0, sin_b,
                            op0=mybir.AluOpType.mult, op1=mybir.AluOpType.mult)
                        nc.vector.tensor_mul(tmp[:, :, 1, :], ps_r[:, :, 0, :], sin_b)
                        # rot = q * cos + tmp
                        nc.vector.tensor_mul(rot_r[:], ps_r[:], cos_b)
                        nc.vector.tensor_add(rot[:], rot[:],
                                             tmp[:].rearrange("p a b c -> p (a b c)"))
                        st = stage.tile([128, HG, 128], F32R, tag=stag)
                        for hb in range(HG):
                            ptr = ps_sm.tile([128, 128], F32R, tag="small")
                            nc.tensor.transpose(ptr[:], rot[:, ts(hb, 128)], ident[:])
                            nc.scalar.copy(st[:, hb, :], ptr[:])
                        nc.sync.dma_start(spill_r[tb // 4][:, :, ts(tb % 4, 128)], st[:])

            # ---------- Phase B+C: attention + output projection -----------
            with ExitStack() as bctx:
                mpool = bctx.enter_context(tc.tile_pool(name="mpool", bufs=1))
                pt_pool = bctx.enter_context(tc.tile_pool(name="pt", bufs=8))
                nrm = bctx.enter_context(tc.tile_pool(name="nrm", bufs=2))
                att_pool = bctx.enter_context(tc.tile_pool(name="attp", bufs=1))
                ystage = bctx.enter_context(tc.tile_pool(name="ystage", bufs=4))

                msk_sb = mpool.tile([128, 4, 512], F32R)
                nc.sync.dma_start(msk_sb[:], msk_d.ap().rearrange("o p q -> p o q"))
                wo_sb = mpool.tile([128, G, D], F32R, tag="wo")
                nc.sync.dma_start(wo_sb[:], woT.ap().rearrange("(ft p) d -> p ft d", p=128))
                qh_cb, kh_cb = [], []
                for cb in range(4):
                    qh = mpool.tile([128, HG, 512], F32R, tag=f"qh{cb}", name=f"qh{cb}")
                    kh = mpool.tile([128, HG, 512], F32R, tag=f"kh{cb}", name=f"kh{cb}")
                    for h in range(HG):
                        nc.gpsimd.dma_start(qh[:, h, :], q_spill_r[cb][:, h, :])
                        nc.gpsimd.dma_start(kh[:, h, :], k_spill_r[cb][:, h, :])
                    qh_cb.append(qh)
                    kh_cb.append(kh)
                attn_sb = att_pool.tile([128, HG, S], F32R, tag="attn_sb")

                for qb in range(NQB):
                    nkt = 4 * qb + 4
                    for h in range(HG):
                        p_att = ps512.tile([128, 512], F32, tag="ps512")
                        p_den = ps_sm.tile([1, 512], F32, tag="small")
                        for kt in range(nkt):
                            p_s = ps512.tile([128, 512], F32, tag="ps512")
                            nc.tensor.matmul(p_s[:],
                                             kh_cb[kt // 4][:, h, ts(kt % 4, 128)],
                                             qh_cb[qb][:, h, :],
                                             start=True, stop=True)
                            pt = pt_pool.tile([128, 512], F32R, tag="pt")
                            nc.scalar.activation(pt[:], p_s[:],
                                                 mybir.ActivationFunctionType.Exp,
                                                 scale=1.0 / math.sqrt(DH))
                            if kt >= 4 * qb:
                                nc.vector.tensor_mul(pt[:], pt[:],
                                                     msk_sb[:, kt - 4 * qb, :])
                            f = dict(start=(kt == 0), stop=(kt == nkt - 1))
                            nc.tensor.matmul(p_att[:],
                                             vh_cb[kt // 4][:, kt % 4, ts(h, 128)],
                                             pt[:], **f)
                            nc.tensor.matmul(p_den[:], ones[:], pt[:], **f)
                        recip = nrm.tile([1, 512], F32, tag="recip")
                        nc.vector.reciprocal_approx_fast(recip[:], p_den[:])
                        rb = nrm.tile([128, 512], F32, tag="rb")
                        nc.gpsimd.partition_broadcast(rb[:], recip[:])
                        nc.vector.tensor_mul(attn_sb[:, h, ts(qb, 512)],
                                             p_att[:], rb[:])
                    # output projection for this query block
                    for qt in range(4 * qb, 4 * qb + 4):
                        for ddb in range(NQB):
                            py = ps512.tile([128, 512], F32, tag="ps512")
                            for ft in range(G):
                                nc.tensor.matmul(py[:], attn_sb[:, ft, ts(qt, 128)],
                                                 wo_sb[:, ft, ts(ddb, 512)],
                                                 start=(ft == 0), stop=(ft == G - 1))
                            y_sb = ystage.tile([128, 512], F32, tag="ysb")
                            nc.scalar.copy(y_sb[:], py[:])
                            nc.sync.dma_start(y.ap()[ts(qt, 128), ts(ddb, 512)],
                                              y_sb[:])

    nc.compile()
    _cache[key] = nc
    return nc


def _in_maps(hidden_q, Wq, Wk, Wv, Wo):
    xs = (hidden_q.astype(np.float32) / math.sqrt(D))
    xT = [np.ascontiguousarray(xs[b].T) for b in range(B)]     # [D, S] each
    cos_t, sin_t = _rope_tables()
    masks = _mask_tiles()
    wo_s = Wo.astype(np.float32) / math.sqrt(H * DH)
    in_maps = []
    for c in range(8):
        b, g = c // G, c % G
        rows = slice(F * g, F * (g + 1))
        in_maps.append({
            "xT": xT[b],
            "wqT": np.ascontiguousarray(Wq[rows, :].T),
            "wkT": np.ascontiguousarray(Wk[rows, :].T),
            "wvT": np.ascontiguousarray(Wv[rows, :].T),
            "woT": np.ascontiguousarray(wo_s[:, rows].T),
            "cos": cos_t, "sin": sin_t, "masks": masks,
        })
    return in_maps


def kernel(hidden_q, attention_mask, position_bias, Wq, Wk, Wv, Wo):
    hidden_q = np.asarray(hidden_q)
    Wq, Wk, Wv, Wo = (np.asarray(w) for w in (Wq, Wk, Wv, Wo))
    assert hidden_q.shape == (B, S, D)
    in_maps = _in_maps(hidden_q, Wq, Wk, Wv, Wo)
    nc = _build()
    res = run_bass_kernel_spmd(nc, in_maps, core_ids=list(range(8)))
    _cache["last_results"] = res
    out = np.zeros((B, S, D), np.float32)
    for c in range(8):
        out[c // G] += res.results[c]["y"]
    return out



# revision 4
# speedup vs baseline: 1.0006x; 1.0006x over previous
"""Multi-head causal self-attention with RoPE on 8 Trainium2 NeuronCores.

Reference computation (B=2, S=2048, D=2048, H=16, DH=128):
    xs = hidden_q / sqrt(D)
    q,k,v = xs @ {Wq,Wk,Wv}.T        (reshaped to [B,H,S,DH])
    q,k <- RoPE(q,k)
    scores = q @ k.T / sqrt(DH)  (causal masked)
    p = softmax(scores); attn = p @ v
    out = (attn / sqrt(H*DH)) @ Wo.T

Sharding: 8 cores = 2 (batch) x 4 (head-groups of 4 heads).  Each core
computes its head-group's projections, attention and a partial output
projection; the host sums the 4 partials per batch.

v2 design (vs the transpose+spill baseline):
  * All matmul operands fp16: full PE rate with fast-weight-load, so
    LDWEIGHTS (~27ns) hides behind the N=512 matmul stream instead of
    serializing 224ns/matmul as fp32 did.
  * Q^T/K^T are produced directly in [dh, seq] layout by using the
    weight slice as the stationary operand (lhsT) and x^T as the moving
    operand -- no PE transposes, no DRAM spill round-trip.
  * RoPE applied on the [dh, seq] tiles with cross-partition DVE ops
    (rotate-half = partition-shifted reads), tables are [dh, S].
  * Softmax denominators: exp tiles are accumulated on DVE into a
    fp32 running sum; ONE ones-matmul per (head, q-block) instead of
    one per score tile (saves ~150 N=512 PE slots).
  * Everything stays in SBUF between phases; phases A (proj), B (attn),
    C (out-proj) are emitted interleaved per 512-token block so the
    Tensor engine never drains.

Layouts on device (per core):
    xT    [D, S]   fp16 feature-major activations (pre-scaled 1/sqrt(D))
    wqT   [D, 512] fp16 per-group weight slices, transposed
    qT,kT [dh=128, h, S] fp16 (RoPE applied)
    v     [128, 16, 512] fp16 natural token-tile layout
    pt    [keys, queries] fp16 exp(scores) tiles
    attn  [dh=128, h, S] fp16 normalized attention output
    y     [S, D]   fp32 partial output (host sums over 4 groups)
"""

import math
from contextlib import ExitStack

import numpy as np

import concourse.bass as bass
import concourse.mybir as mybir
import concourse.tile as tile
from concourse import bacc
from concourse.bass import ts
from concourse.bass_utils import run_bass_kernel_spmd

B, S, D, H, DH = 2, 2048, 2048, 16, 128
BASE = 10000.0
G = 4              # head-groups (cores per batch)
HG = H // G        # heads per group = 4
F = HG * DH        # features per group = 512
NT = S // 128      # 16 token tiles
NKT = D // 128     # 16 contraction tiles
NQB = S // 512     # 4 query blocks
F32 = mybir.dt.float32
F32R = mybir.dt.float32r
F16 = mybir.dt.float16

_cache = {}


def _rope_tables():
    # [dh=128, S] tables, halves duplicated: cosT[p, s] = cos(s*invfreq[p%64])
    inv_freq = 1.0 / (BASE ** (np.arange(0, DH, 2, dtype=np.float64) / DH))
    t = np.arange(S, dtype=np.float64)
    freqs = np.outer(inv_freq, t)                       # [64, S]
    cosT = np.concatenate([np.cos(freqs), np.cos(freqs)], 0)
    sinT = np.concatenate([np.sin(freqs), np.sin(freqs)], 0)
    return cosT.astype(np.float16), sinT.astype(np.float16)


def _mask_tiles():
    # mask[o][j, q] = 1 if q >= j + 128*o else 0  (diagonal-band tiles)
    o = np.arange(4)[:, None, None]
    j = np.arange(128)[None, :, None]
    q = np.arange(512)[None, None, :]
    return (q >= j + 128 * o).astype(np.float16)        # [4, 128, 512]


def _build(reps=1):
    key = ("nc", reps)
    if key in _cache:
        return _cache[key]
    nc = bacc.Bacc("TRN2", target_bir_lowering=False, debug=False, num_devices=8)

    xT = nc.dram_tensor("xT", [D, S], F16, kind="ExternalInput")
    wqT = nc.dram_tensor("wqT", [D, F], F16, kind="ExternalInput")
    wkT = nc.dram_tensor("wkT", [D, F], F16, kind="ExternalInput")
    wvT = nc.dram_tensor("wvT", [D, F], F16, kind="ExternalInput")
    woT = nc.dram_tensor("woT", [F, D], F16, kind="ExternalInput")
    cos_d = nc.dram_tensor("cos", [128, S], F16, kind="ExternalInput")
    sin_d = nc.dram_tensor("sin", [128, S], F16, kind="ExternalInput")
    msk_d = nc.dram_tensor("masks", [4, 128, 512], F16, kind="ExternalInput")
    y = nc.dram_tensor("y", [S, D], F32, kind="ExternalOutput")

    xT_r = xT.ap().rearrange("(kt p) s -> p kt s", p=128)       # [128, 16, S]
    wqT_r = wqT.ap().rearrange("(kt p) f -> p kt f", p=128)
    wkT_r = wkT.ap().rearrange("(kt p) f -> p kt f", p=128)
    wvT_r = wvT.ap().rearrange("(kt p) f -> p kt f", p=128)
    woT_r = woT.ap().rearrange("(ft p) d -> p ft d", p=128)

    with tile.TileContext(nc) as tc, ExitStack() as ctx:
        const = ctx.enter_context(tc.tile_pool(name="const", bufs=1))
        wpool = ctx.enter_context(tc.tile_pool(name="wpool", bufs=1))
        xpool = ctx.enter_context(tc.tile_pool(name="xpool", bufs=2))
        big = ctx.enter_context(tc.tile_pool(name="big", bufs=1))
        pt_pool = ctx.enter_context(tc.tile_pool(name="pt", bufs=6))
        ps_pool = ctx.enter_context(tc.tile_pool(name="ptsum", bufs=2))
        tmp_pool = ctx.enter_context(tc.tile_pool(name="tmp", bufs=2))
        nrm = ctx.enter_context(tc.tile_pool(name="nrm", bufs=2))
        ystage = ctx.enter_context(tc.tile_pool(name="ystage", bufs=3))
        # PSUM: 2 + 3 + 2 + 1 banks = 8
        psA = ctx.enter_context(tc.tile_pool(name="psA", bufs=2, space="PSUM"))
        psS = ctx.enter_context(tc.tile_pool(name="psS", bufs=3, space="PSUM"))
        psT = ctx.enter_context(tc.tile_pool(name="psT", bufs=2, space="PSUM"))
        psD = ctx.enter_context(tc.tile_pool(name="psD", bufs=1, space="PSUM"))

        ones_f = const.tile([128, 1], F32, tag="ones_f")
        nc.gpsimd.memset(ones_f[:], 1.0)
        ones = const.tile([128, 1], F32R, tag="ones")
        nc.vector.tensor_copy(ones[:], ones_f[:])
        msk_sb = const.tile([128, 4, 512], F16, tag="masks")
        nc.gpsimd.dma_start(msk_sb[:], msk_d.ap().rearrange("o p q -> p o q"))

        # static loads (input queue = gpsimd engine; y-out uses sync)
        wq_sb = wpool.tile([128, NKT, F], F16, tag="wq")
        wk_sb = wpool.tile([128, NKT, F], F16, tag="wk")
        wv_sb = wpool.tile([128, NKT, F], F16, tag="wv")
        wo_sb = wpool.tile([128, G, D], F16, tag="wo")
        cos_sb = wpool.tile([128, S], F16, tag="cos")
        sin_sb = wpool.tile([128, S], F16, tag="sin")
        nc.gpsimd.dma_start(wq_sb[:], wqT_r)
        nc.gpsimd.dma_start(wk_sb[:], wkT_r)
        nc.gpsimd.dma_start(wv_sb[:], wvT_r)
        nc.scalar.dma_start(cos_sb[:], cos_d.ap())
        nc.scalar.dma_start(sin_sb[:], sin_d.ap())
        nc.scalar.dma_start(wo_sb[:], woT_r)

        for _rep in range(reps):
            qT = big.tile([128, HG, S], F16, tag="qT", name="qT")
            kT = big.tile([128, HG, S], F16, tag="kT", name="kT")
            v_sb = big.tile([128, NT, F], F16, tag="v", name="v")
            attn_sb = big.tile([128, HG, S], F16, tag="attn", name="attn")

            x_blocks = {}
            for sb in range(2):
                x_blocks[sb] = xpool.tile([128, NKT, 512], F16, tag="x",
                                          name=f"x{sb}")
                nc.sync.dma_start(x_blocks[sb][:], xT_r[:, :, ts(sb, 512)])

            for sb in range(NQB):
                # ---------------- Phase A: projections + RoPE --------------
                x_sb = x_blocks.pop(sb)
                if sb + 2 < NQB:
                    x_blocks[sb + 2] = xpool.tile([128, NKT, 512], F16,
                                                  tag="x", name=f"x{sb+2}")
                    nc.sync.dma_start(x_blocks[sb + 2][:],
                                      xT_r[:, :, ts(sb + 2, 512)])
                sbs = ts(sb, 512)
                for h in range(HG):
                    for (w_sb, out_t) in ((wq_sb, qT), (wk_sb, kT)):
                        ps = psA.tile([128, 512], F32, tag="psA")
                        for kt in range(NKT):
                            nc.tensor.matmul(ps[:], w_sb[:, kt, ts(h, 128)],
                                             x_sb[:, kt, :],
                                             start=(kt == 0),
                                             stop=(kt == NKT - 1))
                        # RoPE: out = ps*cos + rot_half(ps)*sin
                        tmp = tmp_pool.tile([128, 512], F16, tag="rtmp")
                        nc.vector.scalar_tensor_tensor(
                            tmp[0:64, :], ps[64:128, :], -1.0,
                            sin_sb[0:64, sbs],
                            op0=mybir.AluOpType.mult,
                            op1=mybir.AluOpType.mult)
                        nc.vector.tensor_mul(tmp[64:128, :], ps[0:64, :],
                                             sin_sb[64:128, sbs])
                        dst = out_t[:, h, sbs]
                        nc.vector.tensor_mul(dst, ps[:], cos_sb[:, sbs])
                        nc.vector.tensor_add(dst, dst, tmp[:])
                for st in range(4):
                    ps = psA.tile([128, 512], F32, tag="psA")
                    for kt in range(NKT):
                        nc.tensor.matmul(ps[:], x_sb[:, kt, ts(st, 128)],
                                         wv_sb[:, kt, :],
                                         start=(kt == 0),
                                         stop=(kt == NKT - 1))
                    nc.scalar.copy(v_sb[:, 4 * sb + st, :], ps[:])

                # ---------------- Phase B: attention for q-block sb --------
                qb = sb
                nkt = 4 * qb + 4
                for h in range(HG):
                    p_att = psT.tile([128, 512], F32, tag="psT")
                    pt_sum = ps_pool.tile([128, 512], F32R, tag="ptsum")
                    for kt in range(nkt):
                        p_s = psS.tile([128, 512], F32, tag="psS")
                        nc.tensor.matmul(p_s[:], kT[:, h, ts(kt, 128)],
                                         qT[:, h, ts(qb, 512)],
                                         start=True, stop=True)
                        pt = pt_pool.tile([128, 512], F16, tag="pt")
                        nc.scalar.activation(pt[:], p_s[:],
                                             mybir.ActivationFunctionType.Exp,
                                             scale=1.0 / math.sqrt(DH))
                        if kt >= 4 * qb:
                            nc.vector.tensor_mul(pt[:], pt[:],
                                                 msk_sb[:, kt - 4 * qb, :])
                        nc.tensor.matmul(p_att[:], v_sb[:, kt, ts(h, 128)],
                                         pt[:],
                                         start=(kt == 0),
                                         stop=(kt == nkt - 1))
                        if kt == 0:
                            nc.vector.tensor_copy(pt_sum[:], pt[:])
                        else:
                            nc.vector.tensor_add(pt_sum[:], pt_sum[:], pt[:])
                    p_den = psD.tile([1, 512], F32, tag="psD")
                    nc.tensor.matmul(p_den[:], ones[:], pt_sum[:],
                                     start=True, stop=True)
                    recip = nrm.tile([1, 512], F32, tag="recip")
                    nc.vector.reciprocal_approx_fast(recip[:], p_den[:])
                    rb = nrm.tile([128, 512], F32, tag="rb")
                    nc.gpsimd.partition_broadcast(rb[:], recip[:])
                    nc.vector.tensor_mul(attn_sb[:, h, ts(qb, 512)],
                                         p_att[:], rb[:])

                # ---------------- Phase C: output projection ---------------
                for qt in range(4 * qb, 4 * qb + 4):
                    for db in range(NQB):
                        py = psA.tile([128, 512], F32, tag="psA")
                        for ft in range(G):
                            nc.tensor.matmul(py[:],
                                             attn_sb[:, ft, ts(qt, 128)],
                                             wo_sb[:, ft, ts(db, 512)],
                                             start=(ft == 0),
                                             stop=(ft == G - 1))
                        y_sb = ystage.tile([128, 512], F32, tag="ysb")
                        nc.vector.tensor_copy(y_sb[:], py[:])
                        nc.sync.dma_start(y.ap()[ts(qt, 128), ts(db, 512)],
                                          y_sb[:])

    nc.compile()
    _cache[key] = nc
    return nc


def _in_maps(hidden_q, Wq, Wk, Wv, Wo):
    xs = hidden_q.astype(np.float32) / math.sqrt(D)
    xT = [np.ascontiguousarray(xs[b].T).astype(np.float16) for b in range(B)]
    cos_t, sin_t = _rope_tables()
    masks = _mask_tiles()
    wo_s = Wo.astype(np.float32) / math.sqrt(H * DH)
    in_maps = []
    for c in range(8):
        b, g = c // G, c % G
        rows = slice(F * g, F * (g + 1))
        in_maps.append({
            "xT": xT[b],
            "wqT": np.ascontiguousarray(Wq[rows, :].T).astype(np.float16),
            "wkT": np.ascontiguousarray(Wk[rows, :].T).astype(np.float16),
            "wvT": np.ascontiguousarray(Wv[rows, :].T).astype(np.float16),
            "woT": np.ascontiguousarray(wo_s[:, rows].T).astype(np.float16),
            "cos": cos_t, "sin": sin_t, "masks": masks,
        })
    return in_maps


def kernel(hidden_q, attention_mask, position_bias, Wq, Wk, Wv, Wo):
    hidden_q = np.asarray(hidden_q)
    Wq, Wk, Wv, Wo = (np.asarray(w) for w in (Wq, Wk, Wv, Wo))
    assert hidden_q.shape == (B, S, D)
    in_maps = _in_maps(hidden_q, Wq, Wk, Wv, Wo)
    nc = _build()
    res = run_bass_kernel_spmd(nc, in_maps, core_ids=list(range(8)))
    _cache["last_results"] = res
    out = np.zeros((B, S, D), np.float32)
    for c in range(8):
        out[c // G] += res.results[c]["y"]
    return out


# revision 5
# speedup vs baseline: 1.2363x; 1.2355x over previous
"""Multi-head causal self-attention with RoPE on 8 Trainium2 NeuronCores.

Reference computation (B=2, S=2048, D=2048, H=16, DH=128):
    xs = hidden_q / sqrt(D)
    q,k,v = xs @ {Wq,Wk,Wv}.T        (reshaped to [B,H,S,DH])
    q,k <- RoPE(q,k)
    scores = q @ k.T / sqrt(DH)  (causal masked)
    p = softmax(scores); attn = p @ v
    out = (attn / sqrt(H*DH)) @ Wo.T

Sharding: 8 cores = 2 (batch) x 4 (head-groups of 4 heads).  Each core
computes its head-group's projections, attention and a partial output
projection; the host sums the 4 partials per batch.

v3 design notes (all matmul operands fp16, PSUM accumulation fp32):
  * Q^T/K^T produced directly in [dh, seq] layout (weights stationary,
    x^T moving) -- no PE transposes, no DRAM spill.  RoPE applied with
    cross-partition DVE ops on the way out of PSUM.
  * The causal mask is added to the scores IN PSUM by a second matmul
    that accumulates identity.T @ (-30000 band matrix); exp then
    underflows to exactly 0.  Keeps the DVE out of the softmax chain.
  * Softmax denominators accumulate in a [1,512] PSUM bank via a
    ones-vector matmul per key tile (PE), not DVE adds.
  * Normalization: reciprocal of den broadcast across partitions
    (gpsimd) and one DVE multiply per (head, q-block).
  * y partials are written fp16 (host sums 4 partials per batch in
    fp32); staging copies alternate Vector/Scalar engines.
  * Everything stays in SBUF between phases; phases A (proj), B (attn),
    C (out-proj) interleave per 512-token block so the PE never drains.
"""

import math
from contextlib import ExitStack

import numpy as np

import concourse.bass as bass
import concourse.mybir as mybir
import concourse.tile as tile
from concourse import bacc
from concourse.bass import ts
from concourse.bass_utils import run_bass_kernel_spmd
from concourse.masks import make_identity

B, S, D, H, DH = 2, 2048, 2048, 16, 128
BASE = 10000.0
G = 4              # head-groups (cores per batch)
HG = H // G        # heads per group = 4
F = HG * DH        # features per group = 512
NT = S // 128      # 16 token tiles
NKT = D // 128     # 16 contraction tiles
NQB = S // 512     # 4 query blocks
NEG = -30000.0     # causal-mask bias; exp((s+NEG)/sqrt(DH)) == 0
F32 = mybir.dt.float32
F16 = mybir.dt.float16

_cache = {}


def _rope_tables():
    # [dh=128, S] tables, halves duplicated: cosT[p, s] = cos(s*invfreq[p%64])
    inv_freq = 1.0 / (BASE ** (np.arange(0, DH, 2, dtype=np.float64) / DH))
    t = np.arange(S, dtype=np.float64)
    freqs = np.outer(inv_freq, t)                       # [64, S]
    cosT = np.concatenate([np.cos(freqs), np.cos(freqs)], 0)
    sinT = np.concatenate([np.sin(freqs), np.sin(freqs)], 0)
    return cosT.astype(np.float16), sinT.astype(np.float16)


def _mask_tiles():
    # negmask[o][j, q] = 0 where key j+128*o <= query q, else NEG
    o = np.arange(4)[:, None, None]
    j = np.arange(128)[None, :, None]
    q = np.arange(512)[None, None, :]
    return np.where(q >= j + 128 * o, 0.0, NEG).astype(np.float16)


def _build(reps=1):
    key = ("nc", reps)
    if key in _cache:
        return _cache[key]
    nc = bacc.Bacc("TRN2", target_bir_lowering=False, debug=False, num_devices=8)

    xT = nc.dram_tensor("xT", [D, S], F16, kind="ExternalInput")
    wqT = nc.dram_tensor("wqT", [D, F], F16, kind="ExternalInput")
    wkT = nc.dram_tensor("wkT", [D, F], F16, kind="ExternalInput")
    wvT = nc.dram_tensor("wvT", [D, F], F16, kind="ExternalInput")
    woT = nc.dram_tensor("woT", [F, D], F16, kind="ExternalInput")
    cos_d = nc.dram_tensor("cos", [128, S], F16, kind="ExternalInput")
    sin_d = nc.dram_tensor("sin", [128, S], F16, kind="ExternalInput")
    msk_d = nc.dram_tensor("masks", [4, 128, 512], F16, kind="ExternalInput")
    y = nc.dram_tensor("y", [S, D], F16, kind="ExternalOutput")

    xT_r = xT.ap().rearrange("(kt p) s -> p kt s", p=128)       # [128, 16, S]
    wqT_r = wqT.ap().rearrange("(kt p) f -> p kt f", p=128)
    wkT_r = wkT.ap().rearrange("(kt p) f -> p kt f", p=128)
    wvT_r = wvT.ap().rearrange("(kt p) f -> p kt f", p=128)
    woT_r = woT.ap().rearrange("(ft p) d -> p ft d", p=128)

    with tile.TileContext(nc) as tc, ExitStack() as ctx:
        const = ctx.enter_context(tc.tile_pool(name="const", bufs=1))
        wpool = ctx.enter_context(tc.tile_pool(name="wpool", bufs=1))
        xpool = ctx.enter_context(tc.tile_pool(name="xpool", bufs=2))
        big = ctx.enter_context(tc.tile_pool(name="big", bufs=1))
        pt_pool = ctx.enter_context(tc.tile_pool(name="pt", bufs=6))
        tmp_pool = ctx.enter_context(tc.tile_pool(name="tmp", bufs=2))
        nrm = ctx.enter_context(tc.tile_pool(name="nrm", bufs=2))
        ystage = ctx.enter_context(tc.tile_pool(name="ystage", bufs=4))
        # PSUM: 2 + 3 + 2 + 1 banks = 8
        psA = ctx.enter_context(tc.tile_pool(name="psA", bufs=2, space="PSUM"))
        psS = ctx.enter_context(tc.tile_pool(name="psS", bufs=3, space="PSUM"))
        psT = ctx.enter_context(tc.tile_pool(name="psT", bufs=2, space="PSUM"))
        psD = ctx.enter_context(tc.tile_pool(name="psD", bufs=1, space="PSUM"))

        ones = const.tile([128, 1], F16, tag="ones")
        nc.gpsimd.memset(ones[:], 1.0)
        ident = const.tile([128, 128], F16, tag="ident")
        make_identity(nc, ident[:])
        msk_sb = const.tile([128, 4, 512], F16, tag="masks")
        nc.gpsimd.dma_start(msk_sb[:], msk_d.ap().rearrange("o p q -> p o q"))

        # static loads (inputs on gpsimd/scalar queues; x + y-out on sync)
        wq_sb = wpool.tile([128, NKT, F], F16, tag="wq")
        wk_sb = wpool.tile([128, NKT, F], F16, tag="wk")
        wv_sb = wpool.tile([128, NKT, F], F16, tag="wv")
        wo_sb = wpool.tile([128, G, D], F16, tag="wo")
        cos_sb = wpool.tile([128, S], F16, tag="cos")
        sin_sb = wpool.tile([128, S], F16, tag="sin")
        nc.gpsimd.dma_start(wq_sb[:], wqT_r)
        nc.gpsimd.dma_start(wk_sb[:], wkT_r)
        nc.gpsimd.dma_start(wv_sb[:], wvT_r)
        nc.scalar.dma_start(cos_sb[:], cos_d.ap())
        nc.scalar.dma_start(sin_sb[:], sin_d.ap())
        nc.scalar.dma_start(wo_sb[:], woT_r)

        for _rep in range(reps):
            qT = big.tile([128, HG, S], F16, tag="qT", name="qT")
            kT = big.tile([128, HG, S], F16, tag="kT", name="kT")
            v_sb = big.tile([128, NT, F], F16, tag="v", name="v")
            attn_sb = big.tile([128, HG, S], F16, tag="attn", name="attn")

            x_blocks = {}
            for sb in range(2):
                x_blocks[sb] = xpool.tile([128, NKT, 512], F16, tag="x",
                                          name=f"x{sb}")
                nc.sync.dma_start(x_blocks[sb][:], xT_r[:, :, ts(sb, 512)])

            for sb in range(NQB):
                # ---------------- Phase A: projections + RoPE --------------
                x_sb = x_blocks.pop(sb)
                if sb + 2 < NQB:
                    x_blocks[sb + 2] = xpool.tile([128, NKT, 512], F16,
                                                  tag="x", name=f"x{sb+2}")
                    nc.sync.dma_start(x_blocks[sb + 2][:],
                                      xT_r[:, :, ts(sb + 2, 512)])
                sbs = ts(sb, 512)
                for h in range(HG):
                    for (w_sb, out_t) in ((wq_sb, qT), (wk_sb, kT)):
                        ps = psA.tile([128, 512], F32, tag="psA")
                        for kt in range(NKT):
                            nc.tensor.matmul(ps[:], w_sb[:, kt, ts(h, 128)],
                                             x_sb[:, kt, :],
                                             start=(kt == 0),
                                             stop=(kt == NKT - 1))
                        # RoPE: out = ps*cos + rot_half(ps)*sin
                        tmp = tmp_pool.tile([128, 512], F16, tag="rtmp")
                        nc.vector.scalar_tensor_tensor(
                            tmp[0:64, :], ps[64:128, :], -1.0,
                            sin_sb[0:64, sbs],
                            op0=mybir.AluOpType.mult,
                            op1=mybir.AluOpType.mult)
                        nc.vector.tensor_mul(tmp[64:128, :], ps[0:64, :],
                                             sin_sb[64:128, sbs])
                        dst = out_t[:, h, sbs]
                        nc.vector.tensor_mul(dst, ps[:], cos_sb[:, sbs])
                        nc.vector.tensor_add(dst, dst, tmp[:])
                for st in range(4):
                    ps = psA.tile([128, 512], F32, tag="psA")
                    for kt in range(NKT):
                        nc.tensor.matmul(ps[:], x_sb[:, kt, ts(st, 128)],
                                         wv_sb[:, kt, :],
                                         start=(kt == 0),
                                         stop=(kt == NKT - 1))
                    nc.scalar.copy(v_sb[:, 4 * sb + st, :], ps[:])

                # ---------------- Phase B: attention for q-block sb --------
                qb = sb
                nkt = 4 * qb + 4
                for h in range(HG):
                    p_att = psT.tile([128, 512], F32, tag="psT")
                    p_den = psD.tile([1, 512], F32, tag="psD")
                    for kt in range(nkt):
                        p_s = psS.tile([128, 512], F32, tag="psS")
                        diag = kt >= 4 * qb
                        nc.tensor.matmul(p_s[:], kT[:, h, ts(kt, 128)],
                                         qT[:, h, ts(qb, 512)],
                                         start=True, stop=not diag)
                        if diag:
                            # p_s += I.T @ negmask  (causal bias, exp -> 0)
                            nc.tensor.matmul(p_s[:], ident[:],
                                             msk_sb[:, kt - 4 * qb, :],
                                             start=False, stop=True)
                        pt = pt_pool.tile([128, 512], F16, tag="pt")
                        nc.scalar.activation(pt[:], p_s[:],
                                             mybir.ActivationFunctionType.Exp,
                                             scale=1.0 / math.sqrt(DH))
                        nc.tensor.matmul(p_att[:], v_sb[:, kt, ts(h, 128)],
                                         pt[:],
                                         start=(kt == 0),
                                         stop=(kt == nkt - 1))
                        nc.tensor.matmul(p_den[:], ones[:], pt[:],
                                         start=(kt == 0),
                                         stop=(kt == nkt - 1))
                    recip = nrm.tile([1, 512], F32, tag="recip")
                    nc.vector.reciprocal_approx_fast(recip[:], p_den[:])
                    rb = nrm.tile([128, 512], F32, tag="rb")
                    nc.gpsimd.partition_broadcast(rb[:], recip[:])
                    nc.vector.tensor_mul(attn_sb[:, h, ts(qb, 512)],
                                         p_att[:], rb[:])

                # ---------------- Phase C: output projection ---------------
                for qt in range(4 * qb, 4 * qb + 4):
                    for db in range(NQB):
                        py = psA.tile([128, 512], F32, tag="psA")
                        for ft in range(G):
                            nc.tensor.matmul(py[:],
                                             attn_sb[:, ft, ts(qt, 128)],
                                             wo_sb[:, ft, ts(db, 512)],
                                             start=(ft == 0),
                                             stop=(ft == G - 1))
                        y_sb = ystage.tile([128, 512], F16, tag="ysb")
                        if db % 2 == 0:
                            nc.vector.tensor_copy(y_sb[:], py[:])
                        else:
                            nc.scalar.copy(y_sb[:], py[:])
                        nc.sync.dma_start(y.ap()[ts(qt, 128), ts(db, 512)],
                                          y_sb[:])

    nc.compile()
    _cache[key] = nc
    return nc


def _in_maps(hidden_q, Wq, Wk, Wv, Wo):
    xs = hidden_q.astype(np.float32) / math.sqrt(D)
    xT = [np.ascontiguousarray(xs[b].T).astype(np.float16) for b in range(B)]
    cos_t, sin_t = _rope_tables()
    masks = _mask_tiles()
    wo_s = Wo.astype(np.float32) / math.sqrt(H * DH)
    in_maps = []
    for c in range(8):
        b, g = c // G, c % G
        rows = slice(F * g, F * (g + 1))
        in_maps.append({
            "xT": xT[b],
            "wqT": np.ascontiguousarray(Wq[rows, :].T).astype(np.float16),
            "wkT": np.ascontiguousarray(Wk[rows, :].T).astype(np.float16),
            "wvT": np.ascontiguousarray(Wv[rows, :].T).astype(np.float16),
            "woT": np.ascontiguousarray(wo_s[:, rows].T).astype(np.float16),
            "cos": cos_t, "sin": sin_t, "masks": masks,
        })
    return in_maps


def kernel(hidden_q, attention_mask, position_bias, Wq, Wk, Wv, Wo):
    hidden_q = np.asarray(hidden_q)
    Wq, Wk, Wv, Wo = (np.asarray(w) for w in (Wq, Wk, Wv, Wo))
    assert hidden_q.shape == (B, S, D)
    in_maps = _in_maps(hidden_q, Wq, Wk, Wv, Wo)
    nc = _build()
    res = run_bass_kernel_spmd(nc, in_maps, core_ids=list(range(8)))
    _cache["last_results"] = res
    out = np.zeros((B, S, D), np.float32)
    for c in range(8):
        out[c // G] += res.results[c]["y"]
    return out


# revision 9
# speedup vs baseline: 1.2502x; 1.0113x over previous
"""Multi-head causal self-attention with RoPE on 8 Trainium2 NeuronCores.

Reference computation (B=2, S=2048, D=2048, H=16, DH=128):
    xs = hidden_q / sqrt(D)
    q,k,v = xs @ {Wq,Wk,Wv}.T        (reshaped to [B,H,S,DH])
    q,k <- RoPE(q,k)
    scores = q @ k.T / sqrt(DH)  (causal masked)
    p = softmax(scores); attn = p @ v
    out = (attn / sqrt(H*DH)) @ Wo.T

Sharding: 8 cores = 2 (batch) x 4 (head-groups of 4 heads).  Each core
computes its head-group's projections, attention and a partial output
projection; the host sums the 4 partials per batch.

v3 design notes (all matmul operands fp16, PSUM accumulation fp32):
  * Q^T/K^T produced directly in [dh, seq] layout (weights stationary,
    x^T moving) -- no PE transposes, no DRAM spill.  RoPE applied with
    cross-partition DVE ops on the way out of PSUM.
  * The causal mask is added to the scores IN PSUM by a second matmul
    that accumulates identity.T @ (-30000 band matrix); exp then
    underflows to exactly 0.  Keeps the DVE out of the softmax chain.
  * Softmax denominators accumulate in a [1,512] PSUM bank via a
    ones-vector matmul per key tile (PE), not DVE adds.
  * Normalization: reciprocal of den broadcast across partitions
    (gpsimd) and one DVE multiply per (head, q-block).
  * y partials are written fp16 (host sums 4 partials per batch in
    fp32); staging copies alternate Vector/Scalar engines.
  * Everything stays in SBUF between phases; phases A (proj), B (attn),
    C (out-proj) interleave per 512-token block so the PE never drains.
"""

import math
from contextlib import ExitStack

import numpy as np

import concourse.bass as bass
import concourse.mybir as mybir
import concourse.tile as tile
from concourse import bacc
from concourse.bass import ts
from concourse.bass_utils import run_bass_kernel_spmd
from concourse.masks import make_identity

B, S, D, H, DH = 2, 2048, 2048, 16, 128
BASE = 10000.0
G = 4              # head-groups (cores per batch)
HG = H // G        # heads per group = 4
F = HG * DH        # features per group = 512
NT = S // 128      # 16 token tiles
NKT = D // 128     # 16 contraction tiles
NQB = S // 512     # 4 query blocks
NEG = -30000.0     # causal-mask bias; exp((s+NEG)/sqrt(DH)) == 0
F32 = mybir.dt.float32
F16 = mybir.dt.float16

_cache = {}


def _rope_tables():
    # [dh=128, S] tables, halves duplicated: cosT[p, s] = cos(s*invfreq[p%64])
    inv_freq = 1.0 / (BASE ** (np.arange(0, DH, 2, dtype=np.float64) / DH))
    t = np.arange(S, dtype=np.float64)
    freqs = np.outer(inv_freq, t)                       # [64, S]
    cosT = np.concatenate([np.cos(freqs), np.cos(freqs)], 0)
    sinT = np.concatenate([np.sin(freqs), np.sin(freqs)], 0)
    return cosT.astype(np.float16), sinT.astype(np.float16)


def _mask_tiles():
    # negmask[o][j, q] = 0 where key j+128*o <= query q, else NEG
    o = np.arange(4)[:, None, None]
    j = np.arange(128)[None, :, None]
    q = np.arange(512)[None, None, :]
    return np.where(q >= j + 128 * o, 0.0, NEG).astype(np.float16)


def _build(reps=1):
    key = ("nc", reps)
    if key in _cache:
        return _cache[key]
    nc = bacc.Bacc("TRN2", target_bir_lowering=False, debug=False, num_devices=8)

    xT = nc.dram_tensor("xT", [D, S], F16, kind="ExternalInput")
    wqT = nc.dram_tensor("wqT", [D, F], F16, kind="ExternalInput")
    wkT = nc.dram_tensor("wkT", [D, F], F16, kind="ExternalInput")
    wvT = nc.dram_tensor("wvT", [D, F], F16, kind="ExternalInput")
    woT = nc.dram_tensor("woT", [F, D], F16, kind="ExternalInput")
    cos_d = nc.dram_tensor("cos", [128, S], F16, kind="ExternalInput")
    sin_d = nc.dram_tensor("sin", [128, S], F16, kind="ExternalInput")
    msk_d = nc.dram_tensor("masks", [4, 128, 512], F16, kind="ExternalInput")
    y = nc.dram_tensor("y", [S, D], F16, kind="ExternalOutput")

    xT_r = xT.ap().rearrange("(kt p) s -> p kt s", p=128)       # [128, 16, S]
    wqT_r = wqT.ap().rearrange("(kt p) f -> p kt f", p=128)
    wkT_r = wkT.ap().rearrange("(kt p) f -> p kt f", p=128)
    wvT_r = wvT.ap().rearrange("(kt p) f -> p kt f", p=128)
    woT_r = woT.ap().rearrange("(ft p) d -> p ft d", p=128)

    with tile.TileContext(nc) as tc, ExitStack() as ctx:
        const = ctx.enter_context(tc.tile_pool(name="const", bufs=1))
        wpool = ctx.enter_context(tc.tile_pool(name="wpool", bufs=1))
        xpool = ctx.enter_context(tc.tile_pool(name="xpool", bufs=2))
        big = ctx.enter_context(tc.tile_pool(name="big", bufs=1))
        pt_pool = ctx.enter_context(tc.tile_pool(name="pt", bufs=6))
        tmp_pool = ctx.enter_context(tc.tile_pool(name="tmp", bufs=2))
        nrm = ctx.enter_context(tc.tile_pool(name="nrm", bufs=2))
        ystage = ctx.enter_context(tc.tile_pool(name="ystage", bufs=4))
        # PSUM: 2 + 3 + 2 + 1 banks = 8
        psA = ctx.enter_context(tc.tile_pool(name="psA", bufs=2, space="PSUM"))
        psS = ctx.enter_context(tc.tile_pool(name="psS", bufs=3, space="PSUM"))
        psT = ctx.enter_context(tc.tile_pool(name="psT", bufs=2, space="PSUM"))
        psD = ctx.enter_context(tc.tile_pool(name="psD", bufs=1, space="PSUM"))

        ones = const.tile([128, 1], F16, tag="ones")
        nc.gpsimd.memset(ones[:], 1.0)
        ident = const.tile([128, 128], F16, tag="ident")
        make_identity(nc, ident[:])
        msk_sb = const.tile([128, 4, 512], F16, tag="masks")
        nc.scalar.dma_start(msk_sb[:], msk_d.ap().rearrange("o p q -> p o q"))

        # static loads.  The first Q/K chains chase per-kt arrivals, so wq/wk
        # are split per contraction tile; queues: gpsimd=weights, sync=x,
        # scalar=tables (cos/sin needed by the first RoPE, masks/wo later).
        wq_sb = wpool.tile([128, NKT, F], F16, tag="wq")
        wk_sb = wpool.tile([128, NKT, F], F16, tag="wk")
        wv_sb = wpool.tile([128, NKT, F], F16, tag="wv")
        wo_sb = wpool.tile([128, G, D], F16, tag="wo")
        cos_sb = wpool.tile([128, S], F16, tag="cos")
        sin_sb = wpool.tile([128, S], F16, tag="sin")
        for kt in range(NKT):
            nc.gpsimd.dma_start(wq_sb[:, kt, :], wqT_r[:, kt, :])
        nc.scalar.dma_start(cos_sb[:], cos_d.ap())
        nc.scalar.dma_start(sin_sb[:], sin_d.ap())
        for kt in range(NKT):
            nc.gpsimd.dma_start(wk_sb[:, kt, :], wkT_r[:, kt, :])
        nc.gpsimd.dma_start(wv_sb[:], wvT_r)
        nc.scalar.dma_start(wo_sb[:], woT_r)

        for _rep in range(reps):
            qT = big.tile([128, HG, S], F16, tag="qT", name="qT")
            kT = big.tile([128, HG, S], F16, tag="kT", name="kT")
            v_sb = big.tile([128, NT, F], F16, tag="v", name="v")
            attn_sb = big.tile([128, HG, S], F16, tag="attn", name="attn")

            x_blocks = {}
            for sb in range(2):
                x_blocks[sb] = xpool.tile([128, NKT, 512], F16, tag="x",
                                          name=f"x{sb}")
                if sb == 0:
                    for kt in range(NKT):
                        nc.sync.dma_start(x_blocks[0][:, kt, :],
                                          xT_r[:, kt, ts(0, 512)])
                else:
                    nc.sync.dma_start(x_blocks[sb][:],
                                      xT_r[:, :, ts(sb, 512)])

            for sb in range(NQB):
                # ---------------- Phase A: projections + RoPE --------------
                x_sb = x_blocks.pop(sb)
                if sb + 2 < NQB:
                    x_blocks[sb + 2] = xpool.tile([128, NKT, 512], F16,
                                                  tag="x", name=f"x{sb+2}")
                    nc.sync.dma_start(x_blocks[sb + 2][:],
                                      xT_r[:, :, ts(sb + 2, 512)])
                sbs = ts(sb, 512)
                for h in range(HG):
                    for (w_sb, out_t) in ((wq_sb, qT), (wk_sb, kT)):
                        ps = psA.tile([128, 512], F32, tag="psA")
                        for kt in range(NKT):
                            nc.tensor.matmul(ps[:], w_sb[:, kt, ts(h, 128)],
                                             x_sb[:, kt, :],
                                             start=(kt == 0),
                                             stop=(kt == NKT - 1))
                        # RoPE: out = ps*cos + rot_half(ps)*sin
                        tmp = tmp_pool.tile([128, 512], F16, tag="rtmp")
                        nc.vector.scalar_tensor_tensor(
                            tmp[0:64, :], ps[64:128, :], -1.0,
                            sin_sb[0:64, sbs],
                            op0=mybir.AluOpType.mult,
                            op1=mybir.AluOpType.mult)
                        nc.vector.tensor_mul(tmp[64:128, :], ps[0:64, :],
                                             sin_sb[64:128, sbs])
                        dst = out_t[:, h, sbs]
                        nc.vector.tensor_mul(dst, ps[:], cos_sb[:, sbs])
                        nc.vector.tensor_add(dst, dst, tmp[:])
                for st in range(4):
                    ps = psA.tile([128, 512], F32, tag="psA")
                    for kt in range(NKT):
                        nc.tensor.matmul(ps[:], x_sb[:, kt, ts(st, 128)],
                                         wv_sb[:, kt, :],
                                         start=(kt == 0),
                                         stop=(kt == NKT - 1))
                    nc.scalar.copy(v_sb[:, 4 * sb + st, :], ps[:])

                # ---------------- Phase B: attention for q-block sb --------
                qb = sb
                nkt = 4 * qb + 4
                for h in range(HG):
                    p_att = psT.tile([128, 512], F32, tag="psT")
                    p_den = psD.tile([1, 512], F32, tag="psD")
                    # software-pipelined by one kt so no PE instruction waits
                    # on a fresh exp semaphore (queue-head waits block the
                    # LDWEIGHTS pull-ahead and stretch the MM spacing)
                    prev_pt = None

                    def drain(kt, last):
                        nc.tensor.matmul(p_att[:], v_sb[:, kt, ts(h, 128)],
                                         prev_pt[:],
                                         start=(kt == 0), stop=last)
                        nc.tensor.matmul(p_den[:], ones[:], prev_pt[:],
                                         start=(kt == 0), stop=last)

                    for kt in range(nkt):
                        p_s = psS.tile([128, 512], F32, tag="psS")
                        diag = kt >= 4 * qb
                        nc.tensor.matmul(p_s[:], kT[:, h, ts(kt, 128)],
                                         qT[:, h, ts(qb, 512)],
                                         start=True, stop=not diag)
                        if diag:
                            # p_s += I.T @ negmask  (causal bias, exp -> 0)
                            nc.tensor.matmul(p_s[:], ident[:],
                                             msk_sb[:, kt - 4 * qb, :],
                                             start=False, stop=True)
                        if kt > 0:
                            drain(kt - 1, last=False)
                        pt = pt_pool.tile([128, 512], F16, tag="pt")
                        nc.scalar.activation(pt[:], p_s[:],
                                             mybir.ActivationFunctionType.Exp,
                                             scale=1.0 / math.sqrt(DH))
                        prev_pt = pt
                    drain(nkt - 1, last=True)
                    recip = nrm.tile([1, 512], F32, tag="recip")
                    nc.vector.reciprocal_approx_fast(recip[:], p_den[:])
                    rb = nrm.tile([128, 512], F32, tag="rb")
                    nc.gpsimd.partition_broadcast(rb[:], recip[:])
                    nc.vector.tensor_mul(attn_sb[:, h, ts(qb, 512)],
                                         p_att[:], rb[:])

                # ---------------- Phase C: output projection ---------------
                for qt in range(4 * qb, 4 * qb + 4):
                    for db in range(NQB):
                        py = psA.tile([128, 512], F32, tag="psA")
                        for ft in range(G):
                            nc.tensor.matmul(py[:],
                                             attn_sb[:, ft, ts(qt, 128)],
                                             wo_sb[:, ft, ts(db, 512)],
                                             start=(ft == 0),
                                             stop=(ft == G - 1))
                        y_sb = ystage.tile([128, 512], F16, tag="ysb")
                        if db % 2 == 0:
                            nc.vector.tensor_copy(y_sb[:], py[:])
                        else:
                            nc.scalar.copy(y_sb[:], py[:])
                        nc.sync.dma_start(y.ap()[ts(qt, 128), ts(db, 512)],
                                          y_sb[:])

    nc.compile()
    _cache[key] = nc
    return nc


def _in_maps(hidden_q, Wq, Wk, Wv, Wo):
    xs = hidden_q.astype(np.float32) / math.sqrt(D)
    xT = [np.ascontiguousarray(xs[b].T).astype(np.float16) for b in range(B)]
    cos_t, sin_t = _rope_tables()
    masks = _mask_tiles()
    wo_s = Wo.astype(np.float32) / math.sqrt(H * DH)
    in_maps = []
    for c in range(8):
        b, g = c // G, c % G
        rows = slice(F * g, F * (g + 1))
        in_maps.append({
            "xT": xT[b],
            "wqT": np.ascontiguousarray(Wq[rows, :].T).astype(np.float16),
            "wkT": np.ascontiguousarray(Wk[rows, :].T).astype(np.float16),
            "wvT": np.ascontiguousarray(Wv[rows, :].T).astype(np.float16),
            "woT": np.ascontiguousarray(wo_s[:, rows].T).astype(np.float16),
            "cos": cos_t, "sin": sin_t, "masks": masks,
        })
    return in_maps


def kernel(hidden_q, attention_mask, position_bias, Wq, Wk, Wv, Wo):
    hidden_q = np.asarray(hidden_q)
    Wq, Wk, Wv, Wo = (np.asarray(w) for w in (Wq, Wk, Wv, Wo))
    assert hidden_q.shape == (B, S, D)
    in_maps = _in_maps(hidden_q, Wq, Wk, Wv, Wo)
    nc = _build()
    res = run_bass_kernel_spmd(nc, in_maps, core_ids=list(range(8)))
    _cache["last_results"] = res
    out = np.zeros((B, S, D), np.float32)
    for c in range(8):
        out[c // G] += res.results[c]["y"]
    return out
